# revision 19
# baseline (speedup 1.0000x reference)
"""Kalman filter + RTS smoother on 8 Trainium2 NeuronCores.

Structure of the solution
-------------------------
The covariance (Riccati) recursions of this time-invariant Kalman filter
are data-independent and contract geometrically to a steady state
(closed-loop spectral radius ~0.64 here).  In float32 terms the filtered
covariance, the Kalman gain and the smoother gain are exactly constant
after a short transient (n1 ~ 150 of N=4096 steps).  That turns the
sequential problem into:

  * a tiny host-side f64 recursion for the transient gains/covariances,
  * two short truncated matrix convolutions for the means:
        fx_n = sum_{k<KF} C_k y_{n-k}          (causal, filter)
        xs_n = sum_{j<KS} D_j fx_{n+j}         (anticausal, smoother)
    with KF=KS=64 taps (tap matrices decay below 1e-9),
  * broadcast writes of the steady covariances for the bulk of the
    (N,64,64) covariance outputs.

The convolutions and all large output writes run on the 8 NeuronCores,
sharded over the time axis (512 steps per core, with 64-step halos taken
from the zero-padded input).  Each pair of adjacent taps is fused into a
single K=128 PE matmul by storing y (and fx) twice in SBUF, the second
copy shifted by one time step.  The host fixes up only the short
transient head/tail regions where the steady-state formulas don't hold.

If the inputs ever violate the assumptions (NaNs in y, no Riccati
convergence, unexpected shapes), the kernel falls back to an exact
sequential numpy implementation of the reference semantics.
"""

import numpy as np

D = 64
N = 4096
NCORES = 8
T = N // NCORES          # 512 time steps per core
KF = 64                  # filter conv taps
KS = 64                  # smoother conv taps
MARGIN = 16
YW = 640                 # per-core y window width (T + KF + KS halo, padded)
CONV_TOL = 1e-12


def _sym(M):
    return 0.5 * (M + M.T)


# ---------------------------------------------------------------------------
# host-side analysis (float64)
# ---------------------------------------------------------------------------

def _analyze(x0, P0, F, Q, H, R, n_max):
    """Run the data-independent covariance recursions to steady state."""
    d = F.shape[0]
    I = np.eye(d)
    Pp = P0.copy()
    Kts, Pfs, Pps = [], [], []
    n1 = None
    n = 0
    while n < n_max:
        S = H @ Pp @ H.T + R
        try:
            Kt = np.linalg.solve(_sym(S), H @ Pp)
        except np.linalg.LinAlgError:
            return None
        Pf = Pp - Kt @ S.T @ Kt
        Kts.append(Kt)
        Pfs.append(Pf)
        Pps.append(Pp)
        Pp = F @ Pf @ F.T + Q
        if n1 is None and n > 0:
            step = np.max(np.abs(Pfs[-1] - Pfs[-2]))
            scale = max(np.max(np.abs(Pfs[-1])), 1e-30)
            if step / scale < CONV_TOL:
                n1 = min(n + MARGIN, n_max)
        n += 1
        if n1 is not None and n >= n1 + 1:
            break
    if n1 is None or n1 + KF + KS + MARGIN >= n_max:
        return None
    Kt_ss, Pf_ss, Pp_ss = Kts[-1], Pfs[-1], Pps[-1]

    Jt_ss = np.linalg.solve(_sym(F @ Pf_ss @ F.T + Q), F @ Pf_ss)
    A_ss = F @ (I - Kt_ss @ H)
    if (np.linalg.norm(np.linalg.matrix_power(A_ss, KF), 2) > 1e-7
            or np.linalg.norm(np.linalg.matrix_power(Jt_ss, KS), 2) > 1e-7):
        return None

    # filter conv taps C_k
    B_ss = F @ Kt_ss
    IKH = I - Kt_ss @ H
    C = np.empty((KF, d, d))
    C[0] = Kt_ss
    G = B_ss.copy()
    for k in range(1, KF):
        C[k] = IKH @ G
        G = A_ss @ G
    # smoother conv taps D_j
    IJF = I - Jt_ss @ F
    Dt = np.empty((KS, d, d))
    Jp = I.copy()
    for j in range(KS):
        Dt[j] = Jp @ IJF
        Jp = Jt_ss @ Jp

    # backward smoothed covariance: tail transient + steady state
    Ps_tail = [Pf_ss]
    Ps = Pf_ss.copy()
    m2 = None
    m = 0
    while m < n_max:
        Ps = Pf_ss + Jt_ss @ (Ps - Pp_ss) @ Jt_ss.T
        Ps_tail.append(Ps)
        m += 1
        step = np.max(np.abs(Ps_tail[-1] - Ps_tail[-2]))
        if m2 is None and step / max(np.max(np.abs(Ps)), 1e-30) < CONV_TOL:
            m2 = m + MARGIN
        if m2 is not None and m >= m2:
            break
    if m2 is None:
        return None
    Ps_ss = Ps_tail[-1]

    # head transient of smoothed covariance (varying gains below n1)
    Ps_head = np.empty((n1, d, d))
    Ps = Ps_ss.copy()
    for k in range(n1 - 1, -1, -1):
        Pfc = Pps[k + 1] if k + 1 < len(Pps) else Pp_ss
        Jt_k = np.linalg.solve(_sym(Pfc), F @ Pfs[k])
        Ps = Pfs[k] + Jt_k @ (Ps - Pfc) @ Jt_k.T
        Ps_head[k] = Ps

    return dict(
        n1=n1, m2=m2, Kts=Kts, Pfs=np.array(Pfs), Pps=np.array(Pps),
        Kt_ss=Kt_ss, Pf_ss=Pf_ss, Pp_ss=Pp_ss, Jt_ss=Jt_ss,
        C=C, D=Dt, Ps_tail=np.array(Ps_tail), Ps_ss=Ps_ss, Ps_head=Ps_head,
    )


def _smoother_gain(info, F, n):
    if n >= info["n1"]:
        return info["Jt_ss"]
    Pf = info["Pfs"][n]
    Pps = info["Pps"]
    Pfc = Pps[n + 1] if n + 1 < len(Pps) else info["Pp_ss"]
    return np.linalg.solve(_sym(Pfc), F @ Pf)


# ---------------------------------------------------------------------------
# exact sequential fallback (reference semantics, float64 internally)
# ---------------------------------------------------------------------------

def _exact_reference(x, P, y_list, F, Q, H, R):
    Nn, d = y_list.shape
    F, Q, H, R = (a.astype(np.float64) for a in (F, Q, H, R))
    xc = x.astype(np.float64)
    Pc = P.astype(np.float64)
    fx = np.empty((Nn, d))
    fP = np.empty((Nn, d, d))
    for n in range(Nn):
        y = y_list[n].astype(np.float64)
        valid = not np.any(np.isnan(y))
        ys = np.nan_to_num(y, nan=0.0)
        S = H @ Pc @ H.T + R
        Kt = np.linalg.solve(_sym(S), H @ Pc)
        xu = xc + Kt @ (ys - H @ xc)
        Pu = Pc - Kt @ S.T @ Kt
        if not valid:
            xu, Pu = xc, Pc
        fx[n] = xu
        fP[n] = Pu
        xc = F @ xu
        Pc = F @ Pu @ F.T + Q
    sm = np.empty_like(fx)
    sP = np.empty_like(fP)
    xs = fx[Nn - 1].copy()
    Ps = fP[Nn - 1].copy()
    sm[Nn - 1] = xs
    sP[Nn - 1] = Ps
    for n in range(Nn - 2, -1, -1):
        xfc = F @ fx[n]
        Pfc = F @ fP[n] @ F.T + Q
        Jt = np.linalg.solve(_sym(Pfc), F @ fP[n])
        xs = fx[n] + Jt @ (xs - xfc)
        Ps = fP[n] + Jt @ (Ps - Pfc) @ Jt.T
        sm[n] = xs
        sP[n] = Ps
    return (fx.astype(np.float32), fP.astype(np.float32),
            sm.astype(np.float32), sP.astype(np.float32))


# ---------------------------------------------------------------------------
# device kernel (Bass/Tile), compiled once per process
# ---------------------------------------------------------------------------

_BASS_CACHE = {}
LAST_RESULT_INFO = {}


def _build_bass():
    import concourse.bass as bass
    import concourse.mybir as mybir

    dt = mybir.dt.float32
    WFW = (KF // 2) * 64
    WSW = (KS // 2) * 64
    CINW = YW + WFW + WSW
    nc = bass.Bass()
    # all inputs fused into one tensor -> one DMA; raw bass with explicit
    # semaphores (one wait per instruction) because this walrus build
    # rejects multi-wait instructions that Tile's tail drain emits
    cin = nc.declare_dram_parameter("cin", [128, CINW + 2 * D * D], dt, isOutput=False)
    mout = nc.declare_dram_parameter("mout", [128, T], dt, isOutput=True)
    cov = nc.declare_dram_parameter("cov", [T, 2 * D * D], dt, isOutput=True)

    with (
        nc.sbuf_tensor([128, CINW + 2 * D * D], dt) as cint,
        nc.sbuf_tensor([128, T + KS], dt) as fxd,
        nc.sbuf_tensor([128, T], dt) as mt,
        nc.psum_tensor([64, T], dt) as psA,
        nc.psum_tensor([64, KS], dt) as psB,
        nc.psum_tensor([64, T], dt) as psC,
        nc.semaphore("in_sem") as in_sem,
        nc.semaphore("pb_sem") as pb_sem,
        nc.semaphore("pe_sem") as pe_sem,
        nc.semaphore("dve_sem") as dve_sem,
        nc.semaphore("out_sem") as out_sem,
        nc.Block() as block,
    ):
        ydt = cint[:, 0:YW]
        wft = cint[:, YW:YW + WFW]
        wst = cint[:, YW + WFW:CINW]
        pbt = cint[:, CINW:CINW + 2 * D * D]

        @block.sync
        def _(sync):
            # split loads: compute inputs (PE waits on in_sem) and steady-state
            # covariance row (cov writes wait on pb_sem) proceed independently
            sync.dma_start(cint[:, CINW:], cin[:, CINW:]).then_inc(pb_sem, 16)
            sync.dma_start(cint[:, 0:CINW], cin[:, 0:CINW]).then_inc(in_sem, 16)
            sync.wait_ge(pb_sem, 16)
            # bulk covariance output: steady-state [Pf|Ps] row broadcast over
            # all 512 time steps, split over 8 DMAs to spread across HW rings
            for r in range(T // 64):
                sync.dma_start(cov[64 * r:64 * (r + 1), :], pbt[0:64] if r % 2 == 0 else pbt[64:128]).then_inc(out_sem, 16)
            sync.wait_ge(dve_sem, 6)
            sync.dma_start(mout[:], mt[:]).then_inc(out_sem, 16)
            sync.wait_ge(out_sem, 16 * (T // 64 + 1))

        @block.tensor
        def _(tensor):
            tensor.wait_ge(in_sem, 16)
            # filter conv, main columns [0, 512)
            for p in range(KF // 2):
                a = (KF - 1) - 2 * p
                mm = nc.tensor.matmul(psA[:], wft[:, 64 * p:64 * p + 64],
                                      ydt[:, a:a + T],
                                      start=(p == 0), stop=(p == KF // 2 - 1))
            mm.then_inc(pe_sem, 1)
            # filter conv, right halo columns [512, 576)
            for p in range(KF // 2):
                a = (KF - 1) - 2 * p + T
                mm = nc.tensor.matmul(psB[:], wft[:, 64 * p:64 * p + 64],
                                      ydt[:, a:a + KS],
                                      start=(p == 0), stop=(p == KF // 2 - 1))
            mm.then_inc(pe_sem, 1)
            # smoother conv over the duplicated+shifted fx tile
            tensor.wait_ge(dve_sem, 5)
            for p in range(KS // 2):
                mm = nc.tensor.matmul(psC[:], wst[:, 64 * p:64 * p + 64],
                                      fxd[:, 2 * p:2 * p + T],
                                      start=(p == 0), stop=(p == KS // 2 - 1))
            mm.then_inc(pe_sem, 1)

        @block.vector
        def _(vector):
            # fxd[0:64, i] = fx[i]; fxd[64:128, i] = fx[i+1]
            vector.wait_ge(pe_sem, 1)
            nc.vector.tensor_copy(fxd[0:64, 0:T], psA[:]).then_inc(dve_sem, 1)
            nc.vector.tensor_copy(fxd[64:128, 0:T - 1], psA[:, 1:T]).then_inc(dve_sem, 1)
            nc.vector.tensor_copy(mt[0:64, :], psA[:]).then_inc(dve_sem, 1)
            vector.wait_ge(pe_sem, 2)
            nc.vector.tensor_copy(fxd[0:64, T:T + KS], psB[:]).then_inc(dve_sem, 1)
            nc.vector.tensor_copy(fxd[64:128, T - 1:T + KS - 1], psB[:]).then_inc(dve_sem, 1)
            vector.wait_ge(pe_sem, 3)
            nc.vector.tensor_copy(mt[64:128, :], psC[:]).then_inc(dve_sem, 1)
    return nc


def _get_bass():
    if "nc" not in _BASS_CACHE:
        _BASS_CACHE["nc"] = _build_bass()
    return _BASS_CACHE["nc"]


def _ensure_ntff_hook():
    """bass_utils needs antenv.axon_hooks for trace=True under axon; this
    container's antenv lacks it, so register an equivalent shim backed by
    trn_agent_boot's ctypes NTFF driver."""
    import sys
    import types
    try:
        from antenv.axon_hooks import get_axon_ntff_profile_hook  # noqa: F401
        return
    except ImportError:
        pass
    try:
        from trn_agent_boot.trn_boot import _ntff_profile_via_ctypes
        hook = _ntff_profile_via_ctypes("/opt/axon/libaxon_pjrt.so")
    except Exception:
        hook = None
    mod = types.ModuleType("antenv.axon_hooks")
    mod.get_axon_ntff_profile_hook = lambda: hook
    mod.set_axon_ntff_profile_hook = lambda h: None
    if "antenv" not in sys.modules:
        try:
            import antenv  # noqa: F401
        except ImportError:
            pkg = types.ModuleType("antenv")
            pkg.__path__ = []
            sys.modules["antenv"] = pkg
    sys.modules["antenv.axon_hooks"] = mod


def _run_device(in_maps, trace=False):
    from concourse.bass_utils import run_bass_kernel_spmd
    nc = _get_bass()
    if trace:
        try:
            _ensure_ntff_hook()
            res = run_bass_kernel_spmd(nc, in_maps, list(range(NCORES)), trace=True)
            LAST_RESULT_INFO["exec_time_ns"] = res.exec_time_ns
            LAST_RESULT_INFO["profile_json"] = getattr(res, "profile_json", None)
            return res.results
        except Exception as e:  # profiling must never break results
            LAST_RESULT_INFO["trace_error"] = repr(e)
    res = run_bass_kernel_spmd(nc, in_maps, list(range(NCORES)), trace=False)
    LAST_RESULT_INFO["exec_time_ns"] = res.exec_time_ns
    LAST_RESULT_INFO["profile_json"] = getattr(res, "profile_json", None)
    return res.results


# ---------------------------------------------------------------------------
# public entry point
# ---------------------------------------------------------------------------

def kernel(x, P, y_list, F, Q, H, R):
    import os
    x = np.ascontiguousarray(np.asarray(x, dtype=np.float32))
    P = np.ascontiguousarray(np.asarray(P, dtype=np.float32))
    y_list = np.ascontiguousarray(np.asarray(y_list, dtype=np.float32))
    F = np.ascontiguousarray(np.asarray(F, dtype=np.float32))
    Q = np.ascontiguousarray(np.asarray(Q, dtype=np.float32))
    H = np.ascontiguousarray(np.asarray(H, dtype=np.float32))
    R = np.ascontiguousarray(np.asarray(R, dtype=np.float32))

    if y_list.shape != (N, D) or np.isnan(y_list).any():
        return _exact_reference(x, P, y_list, F, Q, H, R)

    F64, Q64, H64, R64 = (a.astype(np.float64) for a in (F, Q, H, R))
    info = _analyze(x.astype(np.float64), P.astype(np.float64),
                    F64, Q64, H64, R64, N)
    if info is None:
        return _exact_reference(x, P, y_list, F, Q, H, R)
    n1 = info["n1"]
    m2 = info["m2"]

    # ---- per-core device inputs ----
    C32 = info["C"].astype(np.float32)
    D32 = info["D"].astype(np.float32)
    wf_np = np.empty((128, (KF // 2) * 64), np.float32)
    ws_np = np.empty((128, (KS // 2) * 64), np.float32)
    for p in range(KF // 2):
        wf_np[0:64, 64 * p:64 * p + 64] = C32[2 * p].T
        wf_np[64:128, 64 * p:64 * p + 64] = C32[2 * p + 1].T
    for p in range(KS // 2):
        ws_np[0:64, 64 * p:64 * p + 64] = D32[2 * p].T
        ws_np[64:128, 64 * p:64 * p + 64] = D32[2 * p + 1].T

    pb_np = np.empty((128, 2 * D * D), np.float32)
    pb_np[:, 0:D * D] = info["Pf_ss"].astype(np.float32).reshape(-1)[None, :]
    pb_np[:, D * D:] = info["Ps_ss"].astype(np.float32).reshape(-1)[None, :]

    # y window per core: yd[0:64, i] = y[base+i], yd[64:128, i] = y[base+i-1]
    ypad = np.zeros((N + 2 * YW, D), np.float32)
    ypad[YW:YW + N] = y_list
    in_maps = []
    for c in range(NCORES):
        base = c * T - (KF - 1)
        w1 = wf_np.shape[1]
        w2 = ws_np.shape[1]
        cin_np = np.empty((128, YW + w1 + w2 + pb_np.shape[1]), np.float32)
        cin_np[0:64, 0:YW] = ypad[YW + base:YW + base + YW].T
        cin_np[64:128, 0:YW] = ypad[YW + base - 1:YW + base + YW - 1].T
        cin_np[:, YW:YW + w1] = wf_np
        cin_np[:, YW + w1:YW + w1 + w2] = ws_np
        cin_np[:, YW + w1 + w2:] = pb_np
        in_maps.append({"cin": cin_np})

    results = _run_device(in_maps, trace=bool(os.environ.get("BASS_KERNEL_TRACE")))

    # ---- assemble full outputs ----
    fx = np.empty((N, D), np.float32)
    xs = np.empty((N, D), np.float32)
    fP = np.empty((N, D, D), np.float32)
    sP = np.empty((N, D, D), np.float32)
    for c in range(NCORES):
        r = results[c]
        fx[c * T:(c + 1) * T] = r["mout"][0:64].T
        xs[c * T:(c + 1) * T] = r["mout"][64:128].T
        fP[c * T:(c + 1) * T] = r["cov"][:, 0:D * D].reshape(T, D, D)
        sP[c * T:(c + 1) * T] = r["cov"][:, D * D:].reshape(T, D, D)

    # ---- host fix-ups of the transient regions (float64 recursions) ----
    m0p = n1 + KF + 8
    # exact filter means for the head
    Kts, Kt_ss = info["Kts"], info["Kt_ss"]
    xp = x.astype(np.float64)
    fx_head = np.empty((m0p, D))
    for n in range(m0p):
        Kt = Kts[n] if n < len(Kts) else Kt_ss
        xu = xp + Kt @ (y_list[n].astype(np.float64) - H64 @ xp)
        fx_head[n] = xu
        xp = F64 @ xu
    fx[:m0p] = fx_head.astype(np.float32)

    fx64 = fx.astype(np.float64)
    # smoothed means: exact tail (terminal condition region)
    t_lo = N - KS - 8
    xs[N - 1] = fx[N - 1]
    carry = fx64[N - 1].copy()
    for n in range(N - 2, t_lo - 1, -1):
        Jt = _smoother_gain(info, F64, n)
        carry = fx64[n] + Jt @ (carry - F64 @ fx64[n])
        xs[n] = carry.astype(np.float32)
    # smoothed means: exact head (time-varying gain region)
    carry = xs[m0p].astype(np.float64)
    for n in range(m0p - 1, -1, -1):
        Jt = _smoother_gain(info, F64, n)
        carry = fx64[n] + Jt @ (carry - F64 @ fx64[n])
        xs[n] = carry.astype(np.float32)

    # covariances: transient head/tail overwrite
    fP[:n1] = info["Pfs"][:n1].astype(np.float32)
    sP[:n1] = info["Ps_head"].astype(np.float32)
    tail = info["Ps_tail"].astype(np.float32)
    for m in range(min(m2 + 1, N)):
        sP[N - 1 - m] = tail[m]

    return fx, fP, xs, sP


# revision 20
# speedup vs baseline: 1.0459x; 1.0459x over previous
"""Kalman filter + RTS smoother on 8 Trainium2 NeuronCores.

Structure of the solution
-------------------------
The covariance (Riccati) recursions of this time-invariant Kalman filter
are data-independent and contract geometrically to a steady state
(closed-loop spectral radius ~0.64 here).  In float32 terms the filtered
covariance, the Kalman gain and the smoother gain are exactly constant
after a short transient (n1 ~ 150 of N=4096 steps).  That turns the
sequential problem into:

  * a tiny host-side f64 recursion for the transient gains/covariances,
  * two short truncated matrix convolutions for the means:
        fx_n = sum_{k<KF} C_k y_{n-k}          (causal, filter)
        xs_n = sum_{j<KS} D_j fx_{n+j}         (anticausal, smoother)
    with KF=KS=64 taps (tap matrices decay below 1e-9),
  * broadcast writes of the steady covariances for the bulk of the
    (N,64,64) covariance outputs.

The convolutions and all large output writes run on the 8 NeuronCores,
sharded over the time axis (512 steps per core, with 64-step halos taken
from the zero-padded input).  Each pair of adjacent taps is fused into a
single K=128 PE matmul by storing y (and fx) twice in SBUF, the second
copy shifted by one time step.  The host fixes up only the short
transient head/tail regions where the steady-state formulas don't hold.

If the inputs ever violate the assumptions (NaNs in y, no Riccati
convergence, unexpected shapes), the kernel falls back to an exact
sequential numpy implementation of the reference semantics.
"""

import numpy as np

D = 64
N = 4096
NCORES = 8
T = N // NCORES          # 512 time steps per core
KF = 64                  # filter conv taps
KS = 64                  # smoother conv taps
MARGIN = 16
YW = 640                 # per-core y window width (T + KF + KS halo, padded)
CONV_TOL = 1e-12


def _sym(M):
    return 0.5 * (M + M.T)


# ---------------------------------------------------------------------------
# host-side analysis (float64)
# ---------------------------------------------------------------------------

def _analyze(x0, P0, F, Q, H, R, n_max):
    """Run the data-independent covariance recursions to steady state."""
    d = F.shape[0]
    I = np.eye(d)
    Pp = P0.copy()
    Kts, Pfs, Pps = [], [], []
    n1 = None
    n = 0
    while n < n_max:
        S = H @ Pp @ H.T + R
        try:
            Kt = np.linalg.solve(_sym(S), H @ Pp)
        except np.linalg.LinAlgError:
            return None
        Pf = Pp - Kt @ S.T @ Kt
        Kts.append(Kt)
        Pfs.append(Pf)
        Pps.append(Pp)
        Pp = F @ Pf @ F.T + Q
        if n1 is None and n > 0:
            step = np.max(np.abs(Pfs[-1] - Pfs[-2]))
            scale = max(np.max(np.abs(Pfs[-1])), 1e-30)
            if step / scale < CONV_TOL:
                n1 = min(n + MARGIN, n_max)
        n += 1
        if n1 is not None and n >= n1 + 1:
            break
    if n1 is None or n1 + KF + KS + MARGIN >= n_max:
        return None
    Kt_ss, Pf_ss, Pp_ss = Kts[-1], Pfs[-1], Pps[-1]

    Jt_ss = np.linalg.solve(_sym(F @ Pf_ss @ F.T + Q), F @ Pf_ss)
    A_ss = F @ (I - Kt_ss @ H)
    if (np.linalg.norm(np.linalg.matrix_power(A_ss, KF), 2) > 1e-7
            or np.linalg.norm(np.linalg.matrix_power(Jt_ss, KS), 2) > 1e-7):
        return None

    # filter conv taps C_k
    B_ss = F @ Kt_ss
    IKH = I - Kt_ss @ H
    C = np.empty((KF, d, d))
    C[0] = Kt_ss
    G = B_ss.copy()
    for k in range(1, KF):
        C[k] = IKH @ G
        G = A_ss @ G
    # smoother conv taps D_j
    IJF = I - Jt_ss @ F
    Dt = np.empty((KS, d, d))
    Jp = I.copy()
    for j in range(KS):
        Dt[j] = Jp @ IJF
        Jp = Jt_ss @ Jp

    # backward smoothed covariance: tail transient + steady state
    Ps_tail = [Pf_ss]
    Ps = Pf_ss.copy()
    m2 = None
    m = 0
    while m < n_max:
        Ps = Pf_ss + Jt_ss @ (Ps - Pp_ss) @ Jt_ss.T
        Ps_tail.append(Ps)
        m += 1
        step = np.max(np.abs(Ps_tail[-1] - Ps_tail[-2]))
        if m2 is None and step / max(np.max(np.abs(Ps)), 1e-30) < CONV_TOL:
            m2 = m + MARGIN
        if m2 is not None and m >= m2:
            break
    if m2 is None:
        return None
    Ps_ss = Ps_tail[-1]

    # head transient of smoothed covariance (varying gains below n1)
    Ps_head = np.empty((n1, d, d))
    Ps = Ps_ss.copy()
    for k in range(n1 - 1, -1, -1):
        Pfc = Pps[k + 1] if k + 1 < len(Pps) else Pp_ss
        Jt_k = np.linalg.solve(_sym(Pfc), F @ Pfs[k])
        Ps = Pfs[k] + Jt_k @ (Ps - Pfc) @ Jt_k.T
        Ps_head[k] = Ps

    return dict(
        n1=n1, m2=m2, Kts=Kts, Pfs=np.array(Pfs), Pps=np.array(Pps),
        Kt_ss=Kt_ss, Pf_ss=Pf_ss, Pp_ss=Pp_ss, Jt_ss=Jt_ss,
        C=C, D=Dt, Ps_tail=np.array(Ps_tail), Ps_ss=Ps_ss, Ps_head=Ps_head,
    )


def _smoother_gain(info, F, n):
    if n >= info["n1"]:
        return info["Jt_ss"]
    Pf = info["Pfs"][n]
    Pps = info["Pps"]
    Pfc = Pps[n + 1] if n + 1 < len(Pps) else info["Pp_ss"]
    return np.linalg.solve(_sym(Pfc), F @ Pf)


# ---------------------------------------------------------------------------
# exact sequential fallback (reference semantics, float64 internally)
# ---------------------------------------------------------------------------

def _exact_reference(x, P, y_list, F, Q, H, R):
    Nn, d = y_list.shape
    F, Q, H, R = (a.astype(np.float64) for a in (F, Q, H, R))
    xc = x.astype(np.float64)
    Pc = P.astype(np.float64)
    fx = np.empty((Nn, d))
    fP = np.empty((Nn, d, d))
    for n in range(Nn):
        y = y_list[n].astype(np.float64)
        valid = not np.any(np.isnan(y))
        ys = np.nan_to_num(y, nan=0.0)
        S = H @ Pc @ H.T + R
        Kt = np.linalg.solve(_sym(S), H @ Pc)
        xu = xc + Kt @ (ys - H @ xc)
        Pu = Pc - Kt @ S.T @ Kt
        if not valid:
            xu, Pu = xc, Pc
        fx[n] = xu
        fP[n] = Pu
        xc = F @ xu
        Pc = F @ Pu @ F.T + Q
    sm = np.empty_like(fx)
    sP = np.empty_like(fP)
    xs = fx[Nn - 1].copy()
    Ps = fP[Nn - 1].copy()
    sm[Nn - 1] = xs
    sP[Nn - 1] = Ps
    for n in range(Nn - 2, -1, -1):
        xfc = F @ fx[n]
        Pfc = F @ fP[n] @ F.T + Q
        Jt = np.linalg.solve(_sym(Pfc), F @ fP[n])
        xs = fx[n] + Jt @ (xs - xfc)
        Ps = fP[n] + Jt @ (Ps - Pfc) @ Jt.T
        sm[n] = xs
        sP[n] = Ps
    return (fx.astype(np.float32), fP.astype(np.float32),
            sm.astype(np.float32), sP.astype(np.float32))


# ---------------------------------------------------------------------------
# device kernel (Bass/Tile), compiled once per process
# ---------------------------------------------------------------------------

_BASS_CACHE = {}
LAST_RESULT_INFO = {}


def _build_bass():
    import concourse.bass as bass
    import concourse.mybir as mybir

    dt = mybir.dt.float32
    WFW = (KF // 2) * 64
    WSW = (KS // 2) * 64
    CINW = YW + WFW + WSW
    nc = bass.Bass()
    # all inputs fused into one tensor -> one DMA; raw bass with explicit
    # semaphores (one wait per instruction) because this walrus build
    # rejects multi-wait instructions that Tile's tail drain emits
    cin = nc.declare_dram_parameter("cin", [128, CINW + 2 * D * D], dt, isOutput=False)
    mout = nc.declare_dram_parameter("mout", [128, T], dt, isOutput=True)
    cov = nc.declare_dram_parameter("cov", [T, 2 * D * D], dt, isOutput=True)

    with (
        nc.sbuf_tensor([128, CINW + 2 * D * D], dt) as cint,
        nc.sbuf_tensor([128, T + KS], dt) as fxd,
        nc.sbuf_tensor([128, T], dt) as mt,
        nc.psum_tensor([64, T], dt) as psA,
        nc.psum_tensor([64, KS], dt) as psB,
        nc.psum_tensor([64, T], dt) as psC,
        nc.semaphore("in_sem") as in_sem,
        nc.semaphore("pb_sem") as pb_sem,
        nc.semaphore("pe_sem") as pe_sem,
        nc.semaphore("dve_sem") as dve_sem,
        nc.semaphore("out_sem") as out_sem,
        nc.Block() as block,
    ):
        ydt = cint[:, 0:YW]
        wft = cint[:, YW:YW + WFW]
        wst = cint[:, YW + WFW:CINW]
        pbt = cint[:, CINW:CINW + 2 * D * D]

        @block.sync
        def _(sync):
            # split loads: compute inputs (PE waits on in_sem) and steady-state
            # covariance row (cov writes wait on pb_sem) proceed independently
            sync.dma_start(cint[:, CINW:], cin[:, CINW:]).then_inc(pb_sem, 16)
            sync.dma_start(cint[:, 0:CINW], cin[:, 0:CINW]).then_inc(in_sem, 16)
            sync.wait_ge(pb_sem, 16)
            # bulk covariance output: steady-state [Pf|Ps] row broadcast over
            # all 512 time steps, split over 4 DMAs (128-partition sources
            # keep all 16 SBUF DMA ports busy) to spread across HW rings
            for r in range(T // 128):
                sync.dma_start(cov[128 * r:128 * (r + 1), :], pbt).then_inc(out_sem, 16)
            sync.wait_ge(dve_sem, 6)
            sync.dma_start(mout[:], mt[:]).then_inc(out_sem, 16)
            sync.wait_ge(out_sem, 16 * (T // 128 + 1))

        @block.tensor
        def _(tensor):
            tensor.wait_ge(in_sem, 16)
            # filter conv, main columns [0, 512)
            for p in range(KF // 2):
                a = (KF - 1) - 2 * p
                mm = nc.tensor.matmul(psA[:], wft[:, 64 * p:64 * p + 64],
                                      ydt[:, a:a + T],
                                      start=(p == 0), stop=(p == KF // 2 - 1))
            mm.then_inc(pe_sem, 1)
            # filter conv, right halo columns [512, 576)
            for p in range(KF // 2):
                a = (KF - 1) - 2 * p + T
                mm = nc.tensor.matmul(psB[:], wft[:, 64 * p:64 * p + 64],
                                      ydt[:, a:a + KS],
                                      start=(p == 0), stop=(p == KF // 2 - 1))
            mm.then_inc(pe_sem, 1)
            # smoother conv over the duplicated+shifted fx tile
            tensor.wait_ge(dve_sem, 5)
            for p in range(KS // 2):
                mm = nc.tensor.matmul(psC[:], wst[:, 64 * p:64 * p + 64],
                                      fxd[:, 2 * p:2 * p + T],
                                      start=(p == 0), stop=(p == KS // 2 - 1))
            mm.then_inc(pe_sem, 1)

        @block.vector
        def _(vector):
            # fxd[0:64, i] = fx[i]; fxd[64:128, i] = fx[i+1]
            vector.wait_ge(pe_sem, 1)
            nc.vector.tensor_copy(fxd[0:64, 0:T], psA[:]).then_inc(dve_sem, 1)
            nc.vector.tensor_copy(fxd[64:128, 0:T - 1], psA[:, 1:T]).then_inc(dve_sem, 1)
            nc.vector.tensor_copy(mt[0:64, :], psA[:]).then_inc(dve_sem, 1)
            vector.wait_ge(pe_sem, 2)
            nc.vector.tensor_copy(fxd[0:64, T:T + KS], psB[:]).then_inc(dve_sem, 1)
            nc.vector.tensor_copy(fxd[64:128, T - 1:T + KS - 1], psB[:]).then_inc(dve_sem, 1)
            vector.wait_ge(pe_sem, 3)
            nc.vector.tensor_copy(mt[64:128, :], psC[:]).then_inc(dve_sem, 1)
    return nc


def _get_bass():
    if "nc" not in _BASS_CACHE:
        _BASS_CACHE["nc"] = _build_bass()
    return _BASS_CACHE["nc"]


def _ensure_ntff_hook():
    """bass_utils needs antenv.axon_hooks for trace=True under axon; this
    container's antenv lacks it, so register an equivalent shim backed by
    trn_agent_boot's ctypes NTFF driver."""
    import sys
    import types
    try:
        from antenv.axon_hooks import get_axon_ntff_profile_hook  # noqa: F401
        return
    except ImportError:
        pass
    try:
        from trn_agent_boot.trn_boot import _ntff_profile_via_ctypes
        hook = _ntff_profile_via_ctypes("/opt/axon/libaxon_pjrt.so")
    except Exception:
        hook = None
    mod = types.ModuleType("antenv.axon_hooks")
    mod.get_axon_ntff_profile_hook = lambda: hook
    mod.set_axon_ntff_profile_hook = lambda h: None
    if "antenv" not in sys.modules:
        try:
            import antenv  # noqa: F401
        except ImportError:
            pkg = types.ModuleType("antenv")
            pkg.__path__ = []
            sys.modules["antenv"] = pkg
    sys.modules["antenv.axon_hooks"] = mod


def _run_device(in_maps, trace=False):
    from concourse.bass_utils import run_bass_kernel_spmd
    nc = _get_bass()
    if trace:
        try:
            _ensure_ntff_hook()
            res = run_bass_kernel_spmd(nc, in_maps, list(range(NCORES)), trace=True)
            LAST_RESULT_INFO["exec_time_ns"] = res.exec_time_ns
            LAST_RESULT_INFO["profile_json"] = getattr(res, "profile_json", None)
            return res.results
        except Exception as e:  # profiling must never break results
            LAST_RESULT_INFO["trace_error"] = repr(e)
    res = run_bass_kernel_spmd(nc, in_maps, list(range(NCORES)), trace=False)
    LAST_RESULT_INFO["exec_time_ns"] = res.exec_time_ns
    LAST_RESULT_INFO["profile_json"] = getattr(res, "profile_json", None)
    return res.results


# ---------------------------------------------------------------------------
# public entry point
# ---------------------------------------------------------------------------

def kernel(x, P, y_list, F, Q, H, R):
    import os
    x = np.ascontiguousarray(np.asarray(x, dtype=np.float32))
    P = np.ascontiguousarray(np.asarray(P, dtype=np.float32))
    y_list = np.ascontiguousarray(np.asarray(y_list, dtype=np.float32))
    F = np.ascontiguousarray(np.asarray(F, dtype=np.float32))
    Q = np.ascontiguousarray(np.asarray(Q, dtype=np.float32))
    H = np.ascontiguousarray(np.asarray(H, dtype=np.float32))
    R = np.ascontiguousarray(np.asarray(R, dtype=np.float32))

    if y_list.shape != (N, D) or np.isnan(y_list).any():
        return _exact_reference(x, P, y_list, F, Q, H, R)

    F64, Q64, H64, R64 = (a.astype(np.float64) for a in (F, Q, H, R))
    info = _analyze(x.astype(np.float64), P.astype(np.float64),
                    F64, Q64, H64, R64, N)
    if info is None:
        return _exact_reference(x, P, y_list, F, Q, H, R)
    n1 = info["n1"]
    m2 = info["m2"]

    # ---- per-core device inputs ----
    C32 = info["C"].astype(np.float32)
    D32 = info["D"].astype(np.float32)
    wf_np = np.empty((128, (KF // 2) * 64), np.float32)
    ws_np = np.empty((128, (KS // 2) * 64), np.float32)
    for p in range(KF // 2):
        wf_np[0:64, 64 * p:64 * p + 64] = C32[2 * p].T
        wf_np[64:128, 64 * p:64 * p + 64] = C32[2 * p + 1].T
    for p in range(KS // 2):
        ws_np[0:64, 64 * p:64 * p + 64] = D32[2 * p].T
        ws_np[64:128, 64 * p:64 * p + 64] = D32[2 * p + 1].T

    pb_np = np.empty((128, 2 * D * D), np.float32)
    pb_np[:, 0:D * D] = info["Pf_ss"].astype(np.float32).reshape(-1)[None, :]
    pb_np[:, D * D:] = info["Ps_ss"].astype(np.float32).reshape(-1)[None, :]

    # y window per core: yd[0:64, i] = y[base+i], yd[64:128, i] = y[base+i-1]
    ypad = np.zeros((N + 2 * YW, D), np.float32)
    ypad[YW:YW + N] = y_list
    in_maps = []
    for c in range(NCORES):
        base = c * T - (KF - 1)
        w1 = wf_np.shape[1]
        w2 = ws_np.shape[1]
        cin_np = np.empty((128, YW + w1 + w2 + pb_np.shape[1]), np.float32)
        cin_np[0:64, 0:YW] = ypad[YW + base:YW + base + YW].T
        cin_np[64:128, 0:YW] = ypad[YW + base - 1:YW + base + YW - 1].T
        cin_np[:, YW:YW + w1] = wf_np
        cin_np[:, YW + w1:YW + w1 + w2] = ws_np
        cin_np[:, YW + w1 + w2:] = pb_np
        in_maps.append({"cin": cin_np})

    results = _run_device(in_maps, trace=bool(os.environ.get("BASS_KERNEL_TRACE")))

    # ---- assemble full outputs ----
    fx = np.empty((N, D), np.float32)
    xs = np.empty((N, D), np.float32)
    fP = np.empty((N, D, D), np.float32)
    sP = np.empty((N, D, D), np.float32)
    for c in range(NCORES):
        r = results[c]
        fx[c * T:(c + 1) * T] = r["mout"][0:64].T
        xs[c * T:(c + 1) * T] = r["mout"][64:128].T
        fP[c * T:(c + 1) * T] = r["cov"][:, 0:D * D].reshape(T, D, D)
        sP[c * T:(c + 1) * T] = r["cov"][:, D * D:].reshape(T, D, D)

    # ---- host fix-ups of the transient regions (float64 recursions) ----
    m0p = n1 + KF + 8
    # exact filter means for the head
    Kts, Kt_ss = info["Kts"], info["Kt_ss"]
    xp = x.astype(np.float64)
    fx_head = np.empty((m0p, D))
    for n in range(m0p):
        Kt = Kts[n] if n < len(Kts) else Kt_ss
        xu = xp + Kt @ (y_list[n].astype(np.float64) - H64 @ xp)
        fx_head[n] = xu
        xp = F64 @ xu
    fx[:m0p] = fx_head.astype(np.float32)

    fx64 = fx.astype(np.float64)
    # smoothed means: exact tail (terminal condition region)
    t_lo = N - KS - 8
    xs[N - 1] = fx[N - 1]
    carry = fx64[N - 1].copy()
    for n in range(N - 2, t_lo - 1, -1):
        Jt = _smoother_gain(info, F64, n)
        carry = fx64[n] + Jt @ (carry - F64 @ fx64[n])
        xs[n] = carry.astype(np.float32)
    # smoothed means: exact head (time-varying gain region)
    carry = xs[m0p].astype(np.float64)
    for n in range(m0p - 1, -1, -1):
        Jt = _smoother_gain(info, F64, n)
        carry = fx64[n] + Jt @ (carry - F64 @ fx64[n])
        xs[n] = carry.astype(np.float32)

    # covariances: transient head/tail overwrite
    fP[:n1] = info["Pfs"][:n1].astype(np.float32)
    sP[:n1] = info["Ps_head"].astype(np.float32)
    tail = info["Ps_tail"].astype(np.float32)
    for m in range(min(m2 + 1, N)):
        sP[N - 1 - m] = tail[m]

    return fx, fP, xs, sP


# revision 21
# speedup vs baseline: 1.1907x; 1.1384x over previous
"""Kalman filter + RTS smoother on 8 Trainium2 NeuronCores.

Structure of the solution
-------------------------
The covariance (Riccati) recursions of this time-invariant Kalman filter
are data-independent and contract geometrically to a steady state
(closed-loop spectral radius ~0.64 here).  In float32 terms the filtered
covariance, the Kalman gain and the smoother gain are exactly constant
after a short transient (n1 ~ 150 of N=4096 steps).  That turns the
sequential problem into:

  * a tiny host-side f64 recursion for the transient gains/covariances,
  * two short truncated matrix convolutions for the means:
        fx_n = sum_{k<KF} C_k y_{n-k}          (causal, filter)
        xs_n = sum_{j<KS} D_j fx_{n+j}         (anticausal, smoother)
    with KF=KS=64 taps (tap matrices decay below 1e-9),
  * broadcast writes of the steady covariances for the bulk of the
    (N,64,64) covariance outputs.

The convolutions and all large output writes run on the 8 NeuronCores,
sharded over the time axis (512 steps per core, with 64-step halos taken
from the zero-padded input).  Each pair of adjacent taps is fused into a
single K=128 PE matmul by storing y (and fx) twice in SBUF, the second
copy shifted by one time step.  The host fixes up only the short
transient head/tail regions where the steady-state formulas don't hold.

If the inputs ever violate the assumptions (NaNs in y, no Riccati
convergence, unexpected shapes), the kernel falls back to an exact
sequential numpy implementation of the reference semantics.
"""

import numpy as np

D = 64
N = 4096
NCORES = 8
T = N // NCORES          # 512 time steps per core
KF = 64                  # filter conv taps
KS = 64                  # smoother conv taps
MARGIN = 16
YW = 640                 # per-core y window width (T + KF + KS halo, padded)
CONV_TOL = 1e-12


def _sym(M):
    return 0.5 * (M + M.T)


# ---------------------------------------------------------------------------
# host-side analysis (float64)
# ---------------------------------------------------------------------------

def _analyze(x0, P0, F, Q, H, R, n_max):
    """Run the data-independent covariance recursions to steady state."""
    d = F.shape[0]
    I = np.eye(d)
    Pp = P0.copy()
    Kts, Pfs, Pps = [], [], []
    n1 = None
    n = 0
    while n < n_max:
        S = H @ Pp @ H.T + R
        try:
            Kt = np.linalg.solve(_sym(S), H @ Pp)
        except np.linalg.LinAlgError:
            return None
        Pf = Pp - Kt @ S.T @ Kt
        Kts.append(Kt)
        Pfs.append(Pf)
        Pps.append(Pp)
        Pp = F @ Pf @ F.T + Q
        if n1 is None and n > 0:
            step = np.max(np.abs(Pfs[-1] - Pfs[-2]))
            scale = max(np.max(np.abs(Pfs[-1])), 1e-30)
            if step / scale < CONV_TOL:
                n1 = min(n + MARGIN, n_max)
        n += 1
        if n1 is not None and n >= n1 + 1:
            break
    if n1 is None or n1 + KF + KS + MARGIN >= n_max:
        return None
    Kt_ss, Pf_ss, Pp_ss = Kts[-1], Pfs[-1], Pps[-1]

    Jt_ss = np.linalg.solve(_sym(F @ Pf_ss @ F.T + Q), F @ Pf_ss)
    A_ss = F @ (I - Kt_ss @ H)
    if (np.linalg.norm(np.linalg.matrix_power(A_ss, KF), 2) > 1e-7
            or np.linalg.norm(np.linalg.matrix_power(Jt_ss, KS), 2) > 1e-7):
        return None

    # filter conv taps C_k
    B_ss = F @ Kt_ss
    IKH = I - Kt_ss @ H
    C = np.empty((KF, d, d))
    C[0] = Kt_ss
    G = B_ss.copy()
    for k in range(1, KF):
        C[k] = IKH @ G
        G = A_ss @ G
    # smoother conv taps D_j
    IJF = I - Jt_ss @ F
    Dt = np.empty((KS, d, d))
    Jp = I.copy()
    for j in range(KS):
        Dt[j] = Jp @ IJF
        Jp = Jt_ss @ Jp

    # backward smoothed covariance: tail transient + steady state
    Ps_tail = [Pf_ss]
    Ps = Pf_ss.copy()
    m2 = None
    m = 0
    while m < n_max:
        Ps = Pf_ss + Jt_ss @ (Ps - Pp_ss) @ Jt_ss.T
        Ps_tail.append(Ps)
        m += 1
        step = np.max(np.abs(Ps_tail[-1] - Ps_tail[-2]))
        if m2 is None and step / max(np.max(np.abs(Ps)), 1e-30) < CONV_TOL:
            m2 = m + MARGIN
        if m2 is not None and m >= m2:
            break
    if m2 is None:
        return None
    Ps_ss = Ps_tail[-1]

    # head transient of smoothed covariance (varying gains below n1)
    Ps_head = np.empty((n1, d, d))
    Ps = Ps_ss.copy()
    for k in range(n1 - 1, -1, -1):
        Pfc = Pps[k + 1] if k + 1 < len(Pps) else Pp_ss
        Jt_k = np.linalg.solve(_sym(Pfc), F @ Pfs[k])
        Ps = Pfs[k] + Jt_k @ (Ps - Pfc) @ Jt_k.T
        Ps_head[k] = Ps

    return dict(
        n1=n1, m2=m2, Kts=Kts, Pfs=np.array(Pfs), Pps=np.array(Pps),
        Kt_ss=Kt_ss, Pf_ss=Pf_ss, Pp_ss=Pp_ss, Jt_ss=Jt_ss,
        C=C, D=Dt, Ps_tail=np.array(Ps_tail), Ps_ss=Ps_ss, Ps_head=Ps_head,
    )


def _smoother_gain(info, F, n):
    if n >= info["n1"]:
        return info["Jt_ss"]
    Pf = info["Pfs"][n]
    Pps = info["Pps"]
    Pfc = Pps[n + 1] if n + 1 < len(Pps) else info["Pp_ss"]
    return np.linalg.solve(_sym(Pfc), F @ Pf)


# ---------------------------------------------------------------------------
# exact sequential fallback (reference semantics, float64 internally)
# ---------------------------------------------------------------------------

def _exact_reference(x, P, y_list, F, Q, H, R):
    Nn, d = y_list.shape
    F, Q, H, R = (a.astype(np.float64) for a in (F, Q, H, R))
    xc = x.astype(np.float64)
    Pc = P.astype(np.float64)
    fx = np.empty((Nn, d))
    fP = np.empty((Nn, d, d))
    for n in range(Nn):
        y = y_list[n].astype(np.float64)
        valid = not np.any(np.isnan(y))
        ys = np.nan_to_num(y, nan=0.0)
        S = H @ Pc @ H.T + R
        Kt = np.linalg.solve(_sym(S), H @ Pc)
        xu = xc + Kt @ (ys - H @ xc)
        Pu = Pc - Kt @ S.T @ Kt
        if not valid:
            xu, Pu = xc, Pc
        fx[n] = xu
        fP[n] = Pu
        xc = F @ xu
        Pc = F @ Pu @ F.T + Q
    sm = np.empty_like(fx)
    sP = np.empty_like(fP)
    xs = fx[Nn - 1].copy()
    Ps = fP[Nn - 1].copy()
    sm[Nn - 1] = xs
    sP[Nn - 1] = Ps
    for n in range(Nn - 2, -1, -1):
        xfc = F @ fx[n]
        Pfc = F @ fP[n] @ F.T + Q
        Jt = np.linalg.solve(_sym(Pfc), F @ fP[n])
        xs = fx[n] + Jt @ (xs - xfc)
        Ps = fP[n] + Jt @ (Ps - Pfc) @ Jt.T
        sm[n] = xs
        sP[n] = Ps
    return (fx.astype(np.float32), fP.astype(np.float32),
            sm.astype(np.float32), sP.astype(np.float32))


# ---------------------------------------------------------------------------
# device kernel (Bass/Tile), compiled once per process
# ---------------------------------------------------------------------------

_BASS_CACHE = {}
LAST_RESULT_INFO = {}


def _build_bass():
    import concourse.bass as bass
    import concourse.mybir as mybir

    dt = mybir.dt.float32
    WFW = (KF // 2) * 64
    WSW = (KS // 2) * 64
    CINW = YW + WFW + WSW
    nc = bass.Bass()
    # all inputs fused into one tensor -> one DMA; raw bass with explicit
    # semaphores (one wait per instruction) because this walrus build
    # rejects multi-wait instructions that Tile's tail drain emits
    cin = nc.declare_dram_parameter("cin", [128, CINW + 2 * D * D], dt, isOutput=False)
    mout = nc.declare_dram_parameter("mout", [128, T], dt, isOutput=True)
    cov = nc.declare_dram_parameter("cov", [T, 2 * D * D], dt, isOutput=True)

    with (
        nc.sbuf_tensor([128, CINW + 2 * D * D], dt) as cint,
        nc.sbuf_tensor([128, T + KS], dt) as fxd,
        nc.sbuf_tensor([128, T], dt) as mt,
        nc.psum_tensor([64, T], dt) as psA,
        nc.psum_tensor([64, KS], dt) as psB,
        nc.psum_tensor([64, T], dt) as psC,
        nc.semaphore("in_sem") as in_sem,
        nc.semaphore("pb_sem") as pb_sem,
        nc.semaphore("pe_sem") as pe_sem,
        nc.semaphore("dve_sem") as dve_sem,
        nc.semaphore("out_sem") as out_sem,
        nc.Block() as block,
    ):
        ydt = cint[:, 0:YW]
        wft = cint[:, YW:YW + WFW]
        wst = cint[:, YW + WFW:CINW]
        pbt = cint[:, CINW:CINW + 2 * D * D]

        @block.sync
        def _(sync):
            # split loads: compute inputs (PE waits on in_sem) and steady-state
            # covariance row (cov writes wait on pb_sem) proceed independently
            sync.dma_start(cint[:, 0:CINW], cin[:, 0:CINW]).then_inc(in_sem, 16)
            sync.dma_start(cint[:, CINW:], cin[:, CINW:]).then_inc(pb_sem, 16)
            sync.wait_ge(pb_sem, 16)
            # bulk covariance output: steady-state [Pf|Ps] row broadcast over
            # all 512 time steps, split over 4 DMAs (128-partition sources
            # keep all 16 SBUF DMA ports busy) to spread across HW rings
            for r in range(T // 128):
                sync.dma_start(cov[128 * r:128 * (r + 1), :], pbt).then_inc(out_sem, 16)
            sync.wait_ge(dve_sem, 6)
            sync.dma_start(mout[:], mt[:]).then_inc(out_sem, 16)
            sync.wait_ge(out_sem, 16 * (T // 128 + 1))

        @block.tensor
        def _(tensor):
            tensor.wait_ge(in_sem, 16)
            # filter conv, main columns [0, 512)
            for p in range(KF // 2):
                a = (KF - 1) - 2 * p
                mm = nc.tensor.matmul(psA[:], wft[:, 64 * p:64 * p + 64],
                                      ydt[:, a:a + T],
                                      start=(p == 0), stop=(p == KF // 2 - 1))
            mm.then_inc(pe_sem, 1)
            # filter conv, right halo columns [512, 576)
            for p in range(KF // 2):
                a = (KF - 1) - 2 * p + T
                mm = nc.tensor.matmul(psB[:], wft[:, 64 * p:64 * p + 64],
                                      ydt[:, a:a + KS],
                                      start=(p == 0), stop=(p == KF // 2 - 1))
            mm.then_inc(pe_sem, 1)
            # smoother conv over the duplicated+shifted fx tile
            tensor.wait_ge(dve_sem, 5)
            for p in range(KS // 2):
                mm = nc.tensor.matmul(psC[:], wst[:, 64 * p:64 * p + 64],
                                      fxd[:, 2 * p:2 * p + T],
                                      start=(p == 0), stop=(p == KS // 2 - 1))
            mm.then_inc(pe_sem, 1)

        @block.vector
        def _(vector):
            # fxd[0:64, i] = fx[i]; fxd[64:128, i] = fx[i+1]
            vector.wait_ge(pe_sem, 1)
            nc.vector.tensor_copy(fxd[0:64, 0:T], psA[:]).then_inc(dve_sem, 1)
            nc.vector.tensor_copy(fxd[64:128, 0:T - 1], psA[:, 1:T]).then_inc(dve_sem, 1)
            nc.vector.tensor_copy(mt[0:64, :], psA[:]).then_inc(dve_sem, 1)
            vector.wait_ge(pe_sem, 2)
            nc.vector.tensor_copy(fxd[0:64, T:T + KS], psB[:]).then_inc(dve_sem, 1)
            nc.vector.tensor_copy(fxd[64:128, T - 1:T + KS - 1], psB[:]).then_inc(dve_sem, 1)
            vector.wait_ge(pe_sem, 3)
            nc.vector.tensor_copy(mt[64:128, :], psC[:]).then_inc(dve_sem, 1)
    return nc


def _get_bass():
    if "nc" not in _BASS_CACHE:
        _BASS_CACHE["nc"] = _build_bass()
    return _BASS_CACHE["nc"]


def _ensure_ntff_hook():
    """bass_utils needs antenv.axon_hooks for trace=True under axon; this
    container's antenv lacks it, so register an equivalent shim backed by
    trn_agent_boot's ctypes NTFF driver."""
    import sys
    import types
    try:
        from antenv.axon_hooks import get_axon_ntff_profile_hook  # noqa: F401
        return
    except ImportError:
        pass
    try:
        from trn_agent_boot.trn_boot import _ntff_profile_via_ctypes
        hook = _ntff_profile_via_ctypes("/opt/axon/libaxon_pjrt.so")
    except Exception:
        hook = None
    mod = types.ModuleType("antenv.axon_hooks")
    mod.get_axon_ntff_profile_hook = lambda: hook
    mod.set_axon_ntff_profile_hook = lambda h: None
    if "antenv" not in sys.modules:
        try:
            import antenv  # noqa: F401
        except ImportError:
            pkg = types.ModuleType("antenv")
            pkg.__path__ = []
            sys.modules["antenv"] = pkg
    sys.modules["antenv.axon_hooks"] = mod


def _run_device(in_maps, trace=False):
    from concourse.bass_utils import run_bass_kernel_spmd
    nc = _get_bass()
    if trace:
        try:
            _ensure_ntff_hook()
            res = run_bass_kernel_spmd(nc, in_maps, list(range(NCORES)), trace=True)
            LAST_RESULT_INFO["exec_time_ns"] = res.exec_time_ns
            LAST_RESULT_INFO["profile_json"] = getattr(res, "profile_json", None)
            return res.results
        except Exception as e:  # profiling must never break results
            LAST_RESULT_INFO["trace_error"] = repr(e)
    res = run_bass_kernel_spmd(nc, in_maps, list(range(NCORES)), trace=False)
    LAST_RESULT_INFO["exec_time_ns"] = res.exec_time_ns
    LAST_RESULT_INFO["profile_json"] = getattr(res, "profile_json", None)
    return res.results


# ---------------------------------------------------------------------------
# public entry point
# ---------------------------------------------------------------------------

def kernel(x, P, y_list, F, Q, H, R):
    import os
    x = np.ascontiguousarray(np.asarray(x, dtype=np.float32))
    P = np.ascontiguousarray(np.asarray(P, dtype=np.float32))
    y_list = np.ascontiguousarray(np.asarray(y_list, dtype=np.float32))
    F = np.ascontiguousarray(np.asarray(F, dtype=np.float32))
    Q = np.ascontiguousarray(np.asarray(Q, dtype=np.float32))
    H = np.ascontiguousarray(np.asarray(H, dtype=np.float32))
    R = np.ascontiguousarray(np.asarray(R, dtype=np.float32))

    if y_list.shape != (N, D) or np.isnan(y_list).any():
        return _exact_reference(x, P, y_list, F, Q, H, R)

    F64, Q64, H64, R64 = (a.astype(np.float64) for a in (F, Q, H, R))
    info = _analyze(x.astype(np.float64), P.astype(np.float64),
                    F64, Q64, H64, R64, N)
    if info is None:
        return _exact_reference(x, P, y_list, F, Q, H, R)
    n1 = info["n1"]
    m2 = info["m2"]

    # ---- per-core device inputs ----
    C32 = info["C"].astype(np.float32)
    D32 = info["D"].astype(np.float32)
    wf_np = np.empty((128, (KF // 2) * 64), np.float32)
    ws_np = np.empty((128, (KS // 2) * 64), np.float32)
    for p in range(KF // 2):
        wf_np[0:64, 64 * p:64 * p + 64] = C32[2 * p].T
        wf_np[64:128, 64 * p:64 * p + 64] = C32[2 * p + 1].T
    for p in range(KS // 2):
        ws_np[0:64, 64 * p:64 * p + 64] = D32[2 * p].T
        ws_np[64:128, 64 * p:64 * p + 64] = D32[2 * p + 1].T

    pb_np = np.empty((128, 2 * D * D), np.float32)
    pb_np[:, 0:D * D] = info["Pf_ss"].astype(np.float32).reshape(-1)[None, :]
    pb_np[:, D * D:] = info["Ps_ss"].astype(np.float32).reshape(-1)[None, :]

    # y window per core: yd[0:64, i] = y[base+i], yd[64:128, i] = y[base+i-1]
    ypad = np.zeros((N + 2 * YW, D), np.float32)
    ypad[YW:YW + N] = y_list
    in_maps = []
    for c in range(NCORES):
        base = c * T - (KF - 1)
        w1 = wf_np.shape[1]
        w2 = ws_np.shape[1]
        cin_np = np.empty((128, YW + w1 + w2 + pb_np.shape[1]), np.float32)
        cin_np[0:64, 0:YW] = ypad[YW + base:YW + base + YW].T
        cin_np[64:128, 0:YW] = ypad[YW + base - 1:YW + base + YW - 1].T
        cin_np[:, YW:YW + w1] = wf_np
        cin_np[:, YW + w1:YW + w1 + w2] = ws_np
        cin_np[:, YW + w1 + w2:] = pb_np
        in_maps.append({"cin": cin_np})

    results = _run_device(in_maps, trace=bool(os.environ.get("BASS_KERNEL_TRACE")))

    # ---- assemble full outputs ----
    fx = np.empty((N, D), np.float32)
    xs = np.empty((N, D), np.float32)
    fP = np.empty((N, D, D), np.float32)
    sP = np.empty((N, D, D), np.float32)
    for c in range(NCORES):
        r = results[c]
        fx[c * T:(c + 1) * T] = r["mout"][0:64].T
        xs[c * T:(c + 1) * T] = r["mout"][64:128].T
        fP[c * T:(c + 1) * T] = r["cov"][:, 0:D * D].reshape(T, D, D)
        sP[c * T:(c + 1) * T] = r["cov"][:, D * D:].reshape(T, D, D)

    # ---- host fix-ups of the transient regions (float64 recursions) ----
    m0p = n1 + KF + 8
    # exact filter means for the head
    Kts, Kt_ss = info["Kts"], info["Kt_ss"]
    xp = x.astype(np.float64)
    fx_head = np.empty((m0p, D))
    for n in range(m0p):
        Kt = Kts[n] if n < len(Kts) else Kt_ss
        xu = xp + Kt @ (y_list[n].astype(np.float64) - H64 @ xp)
        fx_head[n] = xu
        xp = F64 @ xu
    fx[:m0p] = fx_head.astype(np.float32)

    fx64 = fx.astype(np.float64)
    # smoothed means: exact tail (terminal condition region)
    t_lo = N - KS - 8
    xs[N - 1] = fx[N - 1]
    carry = fx64[N - 1].copy()
    for n in range(N - 2, t_lo - 1, -1):
        Jt = _smoother_gain(info, F64, n)
        carry = fx64[n] + Jt @ (carry - F64 @ fx64[n])
        xs[n] = carry.astype(np.float32)
    # smoothed means: exact head (time-varying gain region)
    carry = xs[m0p].astype(np.float64)
    for n in range(m0p - 1, -1, -1):
        Jt = _smoother_gain(info, F64, n)
        carry = fx64[n] + Jt @ (carry - F64 @ fx64[n])
        xs[n] = carry.astype(np.float32)

    # covariances: transient head/tail overwrite
    fP[:n1] = info["Pfs"][:n1].astype(np.float32)
    sP[:n1] = info["Ps_head"].astype(np.float32)
    tail = info["Ps_tail"].astype(np.float32)
    for m in range(min(m2 + 1, N)):
        sP[N - 1 - m] = tail[m]

    return fx, fP, xs, sP
